# revision 29
# baseline (speedup 1.0000x reference)
"""AdaptConv2d Trainium2 kernel — 8-core data-parallel (4 samples/core).

Reference semantics (B=32, C=256, H=W=56):
  ch[b,c]  = 1 if (GAP(relu(conv3x3s2(x))) @ cg_fc_w.T + cg_fc_b)[b,c] > 0 else 0
  layer[b] = 1 if (lstm_head(GAP(x)) @ lg_fc_w.T + lg_fc_b)[b] > 0 else 0
  skip[b]  = (layer[b]==0) | (sum_c ch[b,c]==0)
  out      = x                     where skip
           = ch*conv3x3s1p1(x) + (1-ch)*x   otherwise
(the round(sigmoid(relu(z))) in the reference is exactly z>0, since
 sigmoid(0)=0.5 rounds to 0 under round-half-even).

Strategy: each core owns 4 samples. Phase A streams x through SBUF:
accumulates the spatial sums for GAP and speculatively writes out=x.
Phase B computes the (tiny) layer gate on-chip and loads the "any sample
active" bit into registers on all engines. Phase C, inside tc.If, runs
the two convolutions (fp32r matmuls, shifted-window accumulation over
the 3x3 taps with channels as the contraction dim) and the masked blend,
overwriting out rows for active samples. For inputs where every sample
skips (the seeded problem instance), phase C is branched over and the
kernel is a pure I/O + gate pipeline.
"""

import os

import numpy as np
import ml_dtypes  # noqa: F401  (np bfloat16 registration)

import concourse.bass as bass
import concourse.tile as tile
from concourse import bacc, mybir
from concourse.bass_utils import run_bass_kernel_spmd

F32 = mybir.dt.float32
F32R = mybir.dt.float32r

NCORES = 8
NB = 4            # samples per core
C = 256
H = W = 56
S = H * W         # 3136
HP = H + 2        # 58 (padded)
SP = HP * HP      # 3364
NCH = C // 128    # 2 channel chunks
GH = 27           # gate conv output spatial (stride 2, no pad)
RB = 7            # main-conv row blocks (8 rows x 56 cols = 448)
RBROWS = 8
RBN = RBROWS * W  # 448


def _r(ap, pat, **kw):
    return ap.rearrange(pat, **kw)


def _win(view3, r0, rstep, nr, c0, cstep, ncols):
    """Manual strided window [128, nr, ncols] into a [128, HP, HP] view
    (avoids slice end-bound checks for stride-2 windows that end exactly
    at the last element)."""
    a = view3[:, 0:1, 0:1]
    return bass.AP(
        tensor=a.tensor,
        offset=a.offset + r0 * HP + c0,
        ap=[list(a.ap[0]), [rstep * HP, nr], [cstep, ncols]],
    )


def _build_nc():
    nc = bacc.Bacc(
        "TRN2", target_bir_lowering=False, debug=False,
        enable_asserts=False, num_devices=NCORES,
    )
    x_d = nc.dram_tensor("x", [NB, C, H, W], F32, kind="ExternalInput").ap()
    wm_d = nc.dram_tensor("wm", [18, 128, 256], F32, kind="ExternalInput").ap()
    wg_d = nc.dram_tensor("wg", [18, 128, 256], F32, kind="ExternalInput").ap()
    cgb_d = nc.dram_tensor("cgb", [NCH, 128], F32, kind="ExternalInput").ap()
    fcw_d = nc.dram_tensor("fcwT", [NCH, 128, 256], F32, kind="ExternalInput").ap()
    fcb_d = nc.dram_tensor("fcb", [NCH, 128], F32, kind="ExternalInput").ap()
    lgw_d = nc.dram_tensor("lgwT", [NCH, 128, 10], F32, kind="ExternalInput").ap()
    lgb_d = nc.dram_tensor("lgb", [10, 1], F32, kind="ExternalInput").ap()
    wih_d = nc.dram_tensor("wih4", [4, 128, 10], F32, kind="ExternalInput").ap()
    bih_d = nc.dram_tensor("bih4", [10, 4], F32, kind="ExternalInput").ap()
    fw_d = nc.dram_tensor("fwT", [128, 1], F32, kind="ExternalInput").ap()
    fb_d = nc.dram_tensor("fb", [1, 1], F32, kind="ExternalInput").ap()
    out_d = nc.dram_tensor("out", [NB, C, H, W], F32, kind="ExternalOutput").ap()

    with tile.TileContext(nc) as tc:
        _kernel_body(tc, x_d, wm_d, wg_d, cgb_d, fcw_d, fcb_d, lgw_d, lgb_d,
                     wih_d, bih_d, fw_d, fb_d, out_d)
    nc.compile()
    return nc


def _kernel_body(tc, x_d, wm_d, wg_d, cgb_d, fcw_d, fcb_d, lgw_d, lgb_d,
                 wih_d, bih_d, fw_d, fb_d, out_d):
    nc = tc.nc
    from contextlib import ExitStack

    with ExitStack() as ctx:
        consts = ctx.enter_context(tc.tile_pool(name="consts", bufs=1))
        gates = ctx.enter_context(tc.tile_pool(name="gates", bufs=1))

        # ---- constants into SBUF ----
        # (conv weights are loaded inside the If: the skip path never needs
        # them, and keeping them out of phase A frees DMA bandwidth)
        cgb_sb = consts.tile([128, NCH], F32)
        nc.sync.dma_start(cgb_sb, _r(cgb_d, "c p -> p c"))
        fcw_sb = consts.tile([128, NCH, 256], F32)
        nc.sync.dma_start(fcw_sb, _r(fcw_d, "c p f -> p c f"))
        fcb_sb = consts.tile([128, NCH], F32)
        nc.sync.dma_start(fcb_sb, _r(fcb_d, "c p -> p c"))
        lgw_sb = consts.tile([128, NCH, 10], F32)
        nc.sync.dma_start(lgw_sb, _r(lgw_d, "c p f -> p c f"))
        lgb_sb = consts.tile([10, 1], F32)
        nc.sync.dma_start(lgb_sb, lgb_d)
        wih_sb = consts.tile([128, 4, 10], F32)
        nc.sync.dma_start(wih_sb, _r(wih_d, "g p f -> p g f"))
        bih_sb = consts.tile([10, 4], F32)
        nc.sync.dma_start(bih_sb, bih_d)
        fw_sb = consts.tile([128, 1], F32)
        nc.sync.dma_start(fw_sb, fw_d)
        fb_sb = consts.tile([1, 1], F32)
        nc.sync.dma_start(fb_sb, fb_d)
        ones_sb = consts.tile([128, 1], F32)
        nc.vector.memset(ones_sb, 1.0)
        ones_row = consts.tile([1, 128], F32)
        nc.vector.memset(ones_row, 1.0)

        # zero-padded holders for small-K matmul operands
        q_sb = gates.tile([128, NB], F32)     # lstm input, rows 0..9 live
        nc.vector.memset(q_sb, 0.0)
        h_sb = gates.tile([128, NB], F32)     # lstm hidden, rows 0..9 live
        nc.vector.memset(h_sb, 0.0)

        p_sb = gates.tile([128, NCH, NB], F32)   # spatial sums of x
        bits_sb = gates.tile([1, NB], F32)       # per-sample layer bit
        any_sb = gates.tile([1, 1], F32)
        anyi_sb = gates.tile([1, 1], mybir.dt.int32)

        # ---- phase A: stream x in (GAP sums chase the loads), then write
        # the speculative out=x. All 8 input DMAs are queued before any
        # output DMA so the gate decision is ready ~halfway through the
        # I/O stream and the branch resolves under the output tail.
        xf_tiles = {}
        with tc.tile_pool(name="xf", bufs=8) as xf_pool:
            for b in range(NB):
                for c in range(NCH):
                    xf = xf_pool.tile([128, S], F32, name=f"xf{b}_{c}",
                                      tag="xf")
                    src = x_d[b, c * 128:(c + 1) * 128]      # [128, 56, 56]
                    nc.sync.dma_start(_r(xf, "p (h w) -> p h w", h=H), src)
                    nc.vector.reduce_sum(
                        out=p_sb[:, c, b:b + 1], in_=xf,
                        axis=mybir.AxisListType.X,
                    )
                    xf_tiles[b, c] = xf
            for b in range(NB):
                for c in range(NCH):
                    nc.sync.dma_start(
                        out_d[b, c * 128:(c + 1) * 128],
                        _r(xf_tiles[b, c], "p (h w) -> p h w", h=H),
                    )

        # ---- phase B: layer gate (tiny) ----
        with tc.tile_pool(name="psA", bufs=2, space="PSUM") as psA:
            pg = psA.tile([10, NB], F32)
            nc.tensor.matmul(pg, lhsT=lgw_sb[:, 0], rhs=p_sb[:, 0],
                             start=True, stop=False)
            nc.tensor.matmul(pg, lhsT=lgw_sb[:, 1], rhs=p_sb[:, 1],
                             start=False, stop=True)
            nc.scalar.activation(q_sb[0:10, :], pg,
                                 mybir.ActivationFunctionType.Relu,
                                 bias=lgb_sb)
            pl = psA.tile([10, 4 * NB], F32)
            for k in range(4):
                nc.tensor.matmul(pl[:, k * NB:(k + 1) * NB],
                                 lhsT=wih_sb[:, k], rhs=q_sb,
                                 start=True, stop=True)
            sig_i = gates.tile([10, NB], F32)
            nc.scalar.activation(sig_i, pl[:, 0:NB],
                                 mybir.ActivationFunctionType.Sigmoid,
                                 bias=bih_sb[:, 0:1])
            tanh_g = gates.tile([10, NB], F32)
            nc.scalar.activation(tanh_g, pl[:, 2 * NB:3 * NB],
                                 mybir.ActivationFunctionType.Tanh,
                                 bias=bih_sb[:, 2:3])
            c_sb = gates.tile([10, NB], F32)
            nc.vector.tensor_mul(c_sb, sig_i, tanh_g)
            tanh_c = gates.tile([10, NB], F32)
            nc.scalar.activation(tanh_c, c_sb,
                                 mybir.ActivationFunctionType.Tanh)
            sig_o = gates.tile([10, NB], F32)
            nc.scalar.activation(sig_o, pl[:, 3 * NB:4 * NB],
                                 mybir.ActivationFunctionType.Sigmoid,
                                 bias=bih_sb[:, 3:4])
            nc.vector.tensor_mul(h_sb[0:10, :], sig_o, tanh_c)
            py = psA.tile([1, NB], F32)
            nc.tensor.matmul(py, lhsT=fw_sb, rhs=h_sb, start=True, stop=True)
            # layer bit = (y_pre + fb) > 0, as 1.0/0.0
            nc.vector.tensor_scalar(
                out=bits_sb, in0=py, scalar1=fb_sb, scalar2=0.0,
                op0=mybir.AluOpType.add, op1=mybir.AluOpType.is_gt,
            )
            nc.vector.reduce_max(out=any_sb, in_=bits_sb,
                                 axis=mybir.AxisListType.X)
            nc.vector.tensor_copy(out=anyi_sb, in_=any_sb)

        rv = nc.values_load(anyi_sb[0:1, 0:1], skip_runtime_bounds_check=True)

        # ---- phase C: convs + blend, only when some sample is active ----
        # false (skip) path is the fallthrough: the hot not-taken branch
        # avoids the I-cache miss of jumping over the conv body
        with tc.If(rv > 0, preferred_fallthrough_block=False):
            with tc.tile_pool(name="wpool", bufs=1) as wpool, \
                 tc.tile_pool(name="xpad", bufs=2) as xpad_pool, \
                 tc.tile_pool(name="blend", bufs=4) as bpool, \
                 tc.tile_pool(name="gsc", bufs=4) as gsc, \
                 tc.tile_pool(name="psB", bufs=8, space="PSUM") as psB:
                # conv weights: DMA f32 staging, then round-copy to fp32r
                # (PE runs fp32r at full bf16 rate for N>=256; walrus
                # requires the producer to emit fp32r-rounded data)
                wstage = wpool.tile([128, 18, 256], F32, tag="ws")
                nc.sync.dma_start(wstage, _r(wm_d, "t p f -> p t f"))
                wm_sb = wpool.tile([128, 18, 256], F32R)
                nc.vector.tensor_copy(out=wm_sb, in_=wstage)
                wstage2 = wpool.tile([128, 18, 256], F32, tag="ws")
                nc.sync.dma_start(wstage2, _r(wg_d, "t p f -> p t f"))
                wg_sb = wpool.tile([128, 18, 256], F32R)
                nc.vector.tensor_copy(out=wg_sb, in_=wstage2)
                for b in range(NB):
                    _conv_sample(tc, b, x_d, out_d, xpad_pool, bpool, gsc, psB,
                                 wm_sb, wg_sb, cgb_sb, fcw_sb, fcb_sb,
                                 ones_sb, ones_row, bits_sb)


def _conv_sample(tc, b, x_d, out_d, xpad_pool, bpool, gsc, psB,
                 wm_sb, wg_sb, cgb_sb, fcw_sb, fcb_sb, ones_sb, ones_row,
                 bits_sb):
    nc = tc.nc

    # padded x in SBUF: f32 copy (exact, for the blend) + fp32r rounded
    # copy (PE operand) per channel chunk
    xpad = []
    xpad_r = []
    for c in range(NCH):
        xp = xpad_pool.tile([128, SP], F32, tag="xp")
        xpv = _r(xp, "p (h w) -> p h w", h=HP)
        # zero the pad border (rows 0 and 57, cols 0 and 57)
        nc.vector.memset(xpv[:, 0, :], 0.0)
        nc.vector.memset(xpv[:, HP - 1, :], 0.0)
        nc.vector.memset(xpv[:, 1:HP - 1, 0:1], 0.0)
        nc.vector.memset(xpv[:, 1:HP - 1, HP - 1:HP], 0.0)
        nc.sync.dma_start(xpv[:, 1:1 + H, 1:1 + W],
                          x_d[b, c * 128:(c + 1) * 128])
        xpr = xpad_pool.tile([128, SP], F32R, tag="xpr")
        nc.vector.tensor_copy(out=xpr, in_=xp)
        xpad.append(xpv)
        xpad_r.append(_r(xpr, "p (h w) -> p h w", h=HP))

    # ---- channel-gate conv (3x3 stride2 valid) + GAP + fc ----
    # out rows split 14+13 so every matmul has N>=256 (fp32r full speed).
    # fp32r ISA requires an even innermost moving count, so compute 28
    # columns per row (col 27 hits in-bounds garbage) and reduce over the
    # valid 27 only.
    GHW = GH + 1  # 28
    g3 = gsc.tile([128, NCH, 2], F32)
    for cc in range(NCH):
        for rg, (y0, nr) in enumerate(((0, 14), (14, 13))):
            pgc = psB.tile([128, nr * GHW], F32, tag="ps")
            for t in range(18):
                pos, cic = divmod(t, 2)
                ky, kx = divmod(pos, 3)
                r0 = 1 + 2 * y0 + ky
                rhs = _win(xpad_r[cic], r0, 2, nr, 1 + kx, 2, GHW)
                nc.tensor.matmul(
                    pgc, lhsT=wg_sb[:, t, cc * 128:(cc + 1) * 128],
                    rhs=rhs,
                    start=(t == 0), stop=(t == 17),
                )
            hsc = gsc.tile([128, 14, GH], F32, tag="hsc")
            nc.scalar.activation(
                hsc[:, :nr, :],
                _r(pgc, "p (r c) -> p r c", c=GHW)[:, :, 0:GH],
                mybir.ActivationFunctionType.Relu,
                bias=cgb_sb[:, cc:cc + 1],
                accum_out=g3[:, cc, rg:rg + 1],
            )
    gsum = gsc.tile([128, NCH], F32, tag="gsum")
    for cc in range(NCH):
        nc.vector.reduce_sum(out=gsum[:, cc:cc + 1], in_=g3[:, cc, :],
                             axis=mybir.AxisListType.X)

    chm = []
    for co in range(NCH):
        pfc = psB.tile([128, 1], F32, tag="ps")
        nc.tensor.matmul(pfc, lhsT=fcw_sb[:, 0, co * 128:(co + 1) * 128],
                         rhs=gsum[:, 0:1], start=True, stop=False)
        nc.tensor.matmul(pfc, lhsT=fcw_sb[:, 1, co * 128:(co + 1) * 128],
                         rhs=gsum[:, 1:2], start=False, stop=True)
        m = gsc.tile([128, 1], F32, tag=f"chm{co}")
        nc.vector.tensor_scalar(
            out=m, in0=pfc, scalar1=fcb_sb[:, co:co + 1], scalar2=0.0,
            op0=mybir.AluOpType.add, op1=mybir.AluOpType.is_gt,
        )
        chm.append(m)

    # chsum > 0 (any channel on), AND with this sample's layer bit
    pcs = psB.tile([1, 1], F32, tag="ps")
    nc.tensor.matmul(pcs, lhsT=ones_sb, rhs=chm[0], start=True, stop=False)
    nc.tensor.matmul(pcs, lhsT=ones_sb, rhs=chm[1], start=False, stop=True)
    ncz = gsc.tile([1, 1], F32, tag="ncz")
    nc.vector.tensor_scalar(
        out=ncz, in0=pcs, scalar1=0.5, scalar2=None,
        op0=mybir.AluOpType.is_gt,
    )
    nc.vector.tensor_mul(ncz, ncz, bits_sb[:, b:b + 1])
    pbc = psB.tile([128, 1], F32, tag="ps", name="pbc")
    nc.tensor.matmul(pbc, lhsT=ones_row, rhs=ncz, start=True, stop=True)
    mp = []
    for co in range(NCH):
        m2 = gsc.tile([128, 1], F32, tag=f"mp{co}")
        nc.vector.tensor_mul(m2, chm[co], pbc)
        mp.append(m2)

    # ---- main conv (3x3 stride1 pad1) + masked blend ----
    for co in range(NCH):
        ptiles = [psB.tile([128, RBN], F32, tag="ps", name=f"pmain{rb}")
                  for rb in range(RB)]
        for t in range(18):
            pos, cic = divmod(t, 2)
            ky, kx = divmod(pos, 3)
            lhsT = wm_sb[:, t, co * 128:(co + 1) * 128]
            for rb in range(RB):
                r0 = rb * RBROWS + ky
                rhs = xpad_r[cic][:, r0:r0 + RBROWS, kx:kx + W]
                nc.tensor.matmul(
                    ptiles[rb], lhsT=lhsT, rhs=rhs,
                    start=(t == 0), stop=(t == 17),
                )
        for rb in range(RB):
            xrows = xpad[co][:, 1 + rb * RBROWS:1 + (rb + 1) * RBROWS, 1:1 + W]
            d = bpool.tile([128, RBROWS, W], F32, tag="d")
            nc.vector.tensor_tensor(d, ptiles[rb], xrows,
                                    mybir.AluOpType.subtract)
            o = bpool.tile([128, RBROWS, W], F32, tag="o")
            nc.vector.scalar_tensor_tensor(
                out=o, in0=d, scalar=mp[co], in1=xrows,
                op0=mybir.AluOpType.mult, op1=mybir.AluOpType.add,
            )
            nc.sync.dma_start(
                out_d[b, co * 128:(co + 1) * 128,
                      rb * RBROWS:(rb + 1) * RBROWS, :],
                o,
            )


# ---------------------------------------------------------------- host side

_NC_CACHE = None


def _get_nc():
    global _NC_CACHE
    if _NC_CACHE is None:
        _NC_CACHE = _build_nc()
    return _NC_CACHE


def _prep_weights(inp):
    f = np.float32
    conv_w = np.asarray(inp["conv_w"], f)
    cg_conv_w = np.asarray(inp["cg_conv_w"], f)
    wm = np.ascontiguousarray(
        conv_w.transpose(2, 3, 1, 0).reshape(9, 2, 128, 256).reshape(18, 128, 256))
    wg = np.ascontiguousarray(
        cg_conv_w.transpose(2, 3, 1, 0).reshape(9, 2, 128, 256).reshape(18, 128, 256))
    cgb = np.asarray(inp["cg_conv_b"], f).reshape(2, 128)
    # channel-gate GAP is computed on-device as a spatial SUM; fold the
    # 1/(27*27) mean divisor into the fc weight
    fcwT = np.ascontiguousarray(
        (np.asarray(inp["cg_fc_w"], f).T / float(GH * GH)).reshape(2, 128, 256))
    fcb = np.asarray(inp["cg_fc_b"], f).reshape(2, 128)
    lgw = np.asarray(inp["lg_conv_w"], f).reshape(10, 256)
    lgwT = np.ascontiguousarray((lgw.T / float(S)).reshape(2, 128, 10))
    lgb = np.asarray(inp["lg_conv_b"], f).reshape(10, 1)
    w_ih = np.asarray(inp["lstm_w_ih"], f).reshape(4, 10, 10)
    wih4 = np.zeros((4, 128, 10), f)
    wih4[:, :10, :] = w_ih.transpose(0, 2, 1)
    bih4 = np.ascontiguousarray(
        (np.asarray(inp["lstm_b_ih"], f) + np.asarray(inp["lstm_b_hh"], f))
        .reshape(4, 10).T)
    fwT = np.zeros((128, 1), f)
    fwT[:10, 0] = np.asarray(inp["lg_fc_w"], f).reshape(10)
    fb = np.asarray(inp["lg_fc_b"], f).reshape(1, 1)
    return dict(wm=wm, wg=wg, cgb=cgb, fcwT=fcwT, fcb=fcb, lgwT=lgwT,
                lgb=lgb, wih4=wih4, bih4=bih4, fwT=fwT, fb=fb)


def kernel(**inputs):
    x = np.asarray(inputs["x"], np.float32)
    B = x.shape[0]
    assert B == NCORES * NB, f"expected batch {NCORES * NB}, got {B}"
    w = _prep_weights(inputs)
    in_maps = []
    for i in range(NCORES):
        m = dict(w)
        m["x"] = np.ascontiguousarray(x[i * NB:(i + 1) * NB])
        in_maps.append(m)
    nc = _get_nc()
    res = run_bass_kernel_spmd(
        nc, in_maps, core_ids=list(range(NCORES)),
        trace=bool(os.environ.get("ATHENA_TRACE")),
    )
    kernel.last_result = res
    out = np.concatenate([r["out"] for r in res.results], axis=0)
    return out


kernel.last_result = None
